# revision 7
# baseline (speedup 1.0000x reference)
"""TRN2 Bass kernel for nn_CNN_transformer_hr_xyz_41051297415299.

Reference model (B=32, C=512, D=512, H=8, DFF=2048, K=7), per batch element:
    query_in = causal_conv_in(x)                 # conv over last axis t, mixing C
    xn       = LN0(query_in)                     # over t, (x-m)/(std+eps), ddof=1
    q = conv_q(query_in); k = conv_k(xn); v = conv_v(xn)
    heads split the t axis (8 x 64); attention over the C axis
    o  = softmax(q k^T / 8) v   -> (C, D)
    y  = conv_o(o);  h1 = 2y
    hn = LN1(h1)  ==  LN(y) with eps/2
    out = 2 * (relu(hn @ w1 + b1) @ w2 + b2)

Sharding: data-parallel over batch, 4 per NeuronCore, no collectives.
All matmuls in bf16 (rel err ~7e-3 « 2e-2 gate). All inputs packed into
two flat DRAM blobs (bf16 + f32): per-call staging cost scales with param
count and bytes. Output is bf16, upcast on host.

v4: stage-major over all 4 batch elements (weights DMA'd once per conv),
hn^T kept in SBUF (no DRAM round-trip), bias rank-1 matmuls replaced by
fused DVE adds with row-replicated bias tiles, LN fused (Rsqrt absorbs
eps - 1e-6 relative effect), one 4-block exp per attention head,
transposes grouped 4-per-PSUM-bank.

Device layout (per batch element b):
    std layout = [token chunk c (partitions, 4 chunks), feature t (free)]
    T  layout  = [feature t (partitions, 4 chunks), token (free)]
    x, query_in, xn, o_full : std, padded free dim 6+512 (causal left pad)
    qT, kT : T (conv emitted transposed: lhsT=activation window, rhs=weight)
    v_aug  : [token (part), chunk, head, 66] (64 v cols + ones col -> softmax
             denominator accumulates in the same matmul as o = p @ v)
"""
import numpy as np
from contextlib import ExitStack

try:
    import concourse.bass as bass
except ImportError:  # pragma: no cover - path fallback for bare containers
    import sys
    for _p in ("/opt/trn_rl_repo", "/root/.axon_site/_ro/trn_rl_repo"):
        if _p not in sys.path:
            sys.path.insert(0, _p)
    import concourse.bass as bass

import ml_dtypes
import concourse.mybir as mybir
import concourse.tile as tile
from concourse import bacc
from concourse.bass_utils import run_bass_kernel_spmd
from concourse.masks import make_identity

B, C, D, H, DFF, KW = 32, 512, 512, 8, 2048, 7
NCORES = 8
BL = B // NCORES          # 4 batch elements per core
DH = D // H               # 64
PAD = KW - 1              # 6
F32 = mybir.dt.float32
BF16 = mybir.dt.bfloat16
NPBF = ml_dtypes.bfloat16
AF = mybir.ActivationFunctionType
ALU = mybir.AluOpType
BS = list(range(BL))

# ---- packed wblob (bf16) layout: name -> (offset_elems, rows, cols) ----
_WREG = {}
_WOFF = 0


def _wreg(name, rows, cols):
    global _WOFF
    _WREG[name] = (_WOFF, rows, cols)
    _WOFF += rows * cols


for _n in ("win", "wq", "wk", "wv", "wo"):
    for _ci in range(4):
        _wreg(f"{_n}{_ci}", 128, KW * C)
for _ci in range(4):
    _wreg(f"w1_{_ci}", 128, DFF)
for _fc in range(16):
    _wreg(f"w2_{_fc}", 128, D)
for _b in range(BL):
    _wreg(f"x{_b}", 128, 4 * (PAD + D))
WBLOB = _WOFF

# ---- packed fblob (f32) layout ----
_FREG = {}
_FOFF = 0


def _freg(name, rows, cols):
    global _FOFF
    _FREG[name] = (_FOFF, rows, cols)
    _FOFF += rows * cols


_freg("bpp", 128, 36)
for _n in ("ln0g", "ln0b", "ln1g", "ln1b", "bqr", "bkr", "b2r2"):
    _freg(_n, 128, D)
FBLOB = _FOFF


def _conv_w_host(w):
    """(cout, cin, KW) -> (4, 128, KW*512): [ci][p][k*512+cout]."""
    return np.ascontiguousarray(
        w.transpose(1, 2, 0).reshape(4, 128, KW * C).astype(NPBF))


def build_nc(reps=1):
    nc = bacc.Bacc("TRN2", target_bir_lowering=False, debug=False)

    wblob = nc.declare_dram_parameter("wblob", [WBLOB], BF16, isOutput=False)
    fblob = nc.declare_dram_parameter("fblob", [FBLOB], F32, isOutput=False)
    outp = nc.declare_dram_parameter("outp", [BL, C, D], BF16, isOutput=True)

    def wsrc(name):
        off, r, c = _WREG[name]
        return wblob.ap()[off:off + r * c].rearrange("(p t) -> p t", p=r)

    def fsrc(name):
        off, r, c = _FREG[name]
        return fblob.ap()[off:off + r * c].rearrange("(p t) -> p t", p=r)

    with tile.TileContext(nc) as tc, ExitStack() as octx:
        cp = octx.enter_context(tc.tile_pool(name="consts", bufs=1))

        def ctile(name, shape, dtype, src):
            t = cp.tile(shape, dtype, tag=name, name=name)
            nc.sync.dma_start(t[:], src)
            return t

        bpp = ctile("bpp", [128, 36], F32, fsrc("bpp"))
        ln_t = {n: ctile(n, [128, D], F32, fsrc(n))
                for n in ("ln0g", "ln0b", "ln1g", "ln1b", "bqr", "bkr", "b2r2")}
        identb = cp.tile([128, 128], BF16, tag="identb", name="identb")
        make_identity(nc, identb[:])

        hnTp = octx.enter_context(tc.tile_pool(name="hnTp", bufs=4))

        def load_w(pool, wname, label):
            ts = []
            for ci in range(4):
                t = pool.tile([128, KW * C], BF16, tag="w", name=f"{label}{ci}")
                nc.sync.dma_start(t[:], wsrc(f"{wname}{ci}"))
                ts.append(t)
            return ts

        def conv_std(pmm, wt, src, writer):
            """std conv: out[cout, t] accumulated over (cin chunk, tap);
            stationary weight shared by all 4 batch elements."""
            for oc in range(4):
                ps = {b: pmm.tile([128, D], F32, tag="mm", name=f"cs{oc}{b}")
                      for b in BS}
                for ci in range(4):
                    for k in range(KW):
                        lhsT = wt[ci][:, k * C + oc * 128: k * C + oc * 128 + 128]
                        for b in BS:
                            nc.tensor.matmul(
                                ps[b][:], lhsT, src[b][:, ci, k:k + D],
                                start=(ci == 0 and k == 0),
                                stop=(ci == 3 and k == KW - 1))
                for b in BS:
                    writer(b, oc, ps[b])

        def conv_T(pmm, wt, src, brow_t, dst):
            """transposed conv: out[t, cout]; bias added by the DVE drain."""
            for tcn in range(4):
                ps = {b: pmm.tile([128, D], F32, tag="mm", name=f"cT{tcn}{b}")
                      for b in BS}
                for ci in range(4):
                    for k in range(KW):
                        rhs = wt[ci][:, k * C:(k + 1) * C]
                        for b in BS:
                            lhsT = src[b][:, ci, tcn * 128 + k: tcn * 128 + k + 128]
                            nc.tensor.matmul(ps[b][:], lhsT, rhs,
                                             start=(ci == 0 and k == 0),
                                             stop=(ci == 3 and k == KW - 1))
                for b in BS:
                    nc.vector.tensor_add(dst[b][:, tcn, :], ps[b][:], brow_t[:])

        def emit_ln(lnw, stat, src, dst, g_t, b_t, padded_src):
            """LN over free axis; 1/(std+eps) ~= rsqrt(var) (eps ~ 1e-6)."""
            for b in BS:
                for c in range(4):
                    sv = (src[b][:, c, PAD:PAD + D] if padded_src
                          else src[b][:, c, :])
                    sm = stat.tile([128, 1], F32, tag="st", name=f"sm{b}{c}")
                    nc.vector.reduce_sum(sm[:], sv, axis=mybir.AxisListType.X)
                    mn = stat.tile([128, 1], F32, tag="st", name=f"mn{b}{c}")
                    nc.scalar.mul(mn[:], sm[:], 1.0 / D)
                    cent = lnw.tile([128, D], F32, tag="lw", name=f"ce{b}{c}")
                    nc.vector.tensor_scalar(cent[:], sv, mn[:], None,
                                            op0=ALU.subtract)
                    scr = lnw.tile([128, D], F32, tag="lw", name=f"sc{b}{c}")
                    sq = stat.tile([128, 1], F32, tag="st", name=f"sq{b}{c}")
                    nc.scalar.activation(scr[:], cent[:], AF.Square,
                                         accum_out=sq[:])
                    st = stat.tile([128, 1], F32, tag="st", name=f"sd{b}{c}")
                    nc.scalar.activation(st[:], sq[:], AF.Sqrt,
                                         scale=1.0 / (D - 1))
                    iv = stat.tile([128, 1], F32, tag="st", name=f"iv{b}{c}")
                    nc.vector.reciprocal(iv[:], st[:])
                    tmp = lnw.tile([128, D], F32, tag="lw", name=f"tm{b}{c}")
                    nc.vector.scalar_tensor_tensor(
                        tmp[:], in0=cent[:], scalar=iv[:], in1=g_t[:],
                        op0=ALU.mult, op1=ALU.mult)
                    dv = (dst[b][:, c, PAD:PAD + D] if padded_src
                          else dst[b][:, c, :])
                    nc.vector.tensor_add(dv, tmp[:], b_t[:])

        def zero_pads(t):
            nc.gpsimd.memset(t[:, :, 0:PAD], 0.0)

        for _rep in range(reps):
            with ExitStack() as wctx:
                wconv = wctx.enter_context(tc.tile_pool(name="wconv", bufs=8))
                attp = wctx.enter_context(tc.tile_pool(name="attp", bufs=4))
                qT = {b: attp.tile([128, 4, D], BF16, tag="qT", name=f"qT{b}")
                      for b in BS}
                kT = {b: attp.tile([128, 4, D], BF16, tag="kT", name=f"kT{b}")
                      for b in BS}
                vaug = {b: attp.tile([128, 4, H, DH + 2], BF16, tag="va",
                                     name=f"vaug{b}") for b in BS}
                ofull = {b: attp.tile([128, 4, PAD + D], BF16, tag="of",
                                      name=f"of{b}") for b in BS}

                # ---- P1: conv_in, LN0, conv_q, conv_k, conv_v ----
                with ExitStack() as p1:
                    pmm = p1.enter_context(
                        tc.tile_pool(name="pmm", bufs=8, space="PSUM"))
                    qxp = p1.enter_context(tc.tile_pool(name="qxp", bufs=8))
                    lnw = p1.enter_context(tc.tile_pool(name="lnw", bufs=2))
                    stat = p1.enter_context(tc.tile_pool(name="stat", bufs=16))
                    qin = {b: qxp.tile([128, 4, PAD + D], BF16, tag="qx",
                                       name=f"qin{b}") for b in BS}
                    xn = {b: qxp.tile([128, 4, PAD + D], BF16, tag="qx",
                                      name=f"xn{b}") for b in BS}

                    with ExitStack() as sx:
                        xpl = sx.enter_context(tc.tile_pool(name="xpl", bufs=4))
                        x_t = {}
                        for b in BS:
                            x_t[b] = xpl.tile([128, 4, PAD + D], BF16, tag="x",
                                              name=f"x{b}")
                            nc.sync.dma_start(
                                x_t[b][:],
                                wsrc(f"x{b}").rearrange("p (c t) -> p c t", c=4))
                        w_t = load_w(wconv, "win", "win")
                        for b in BS:
                            zero_pads(qin[b])

                        def wr_qin(b, oc, ps):
                            nc.scalar.activation(qin[b][:, oc, PAD:PAD + D],
                                                 ps[:], AF.Identity,
                                                 bias=bpp[:, oc:oc + 1])
                        conv_std(pmm, w_t, x_t, wr_qin)

                    w_t = load_w(wconv, "wq", "wq")     # prefetch
                    for b in BS:
                        zero_pads(xn[b])
                    emit_ln(lnw, stat, qin, xn, ln_t["ln0g"], ln_t["ln0b"],
                            padded_src=True)

                    conv_T(pmm, w_t, qin, ln_t["bqr"], qT)
                    w_t = load_w(wconv, "wk", "wk")
                    conv_T(pmm, w_t, xn, ln_t["bkr"], kT)

                    w_t = load_w(wconv, "wv", "wv")
                    for b in BS:
                        nc.gpsimd.memset(vaug[b][:, :, :, DH:DH + 1], 1.0)
                        nc.gpsimd.memset(vaug[b][:, :, :, DH + 1:DH + 2], 0.0)

                    def wr_v(b, oc, ps):
                        nc.scalar.activation(
                            vaug[b][:, oc, :, 0:DH],
                            ps[:].rearrange("p (h dd) -> p h dd", h=H),
                            AF.Identity, bias=bpp[:, 4 + oc:5 + oc])
                    conv_std(pmm, w_t, xn, wr_v)
                    wo_t = load_w(wconv, "wo", "wo")

                # ---- P2a: attention ----
                with ExitStack() as p2a:
                    psp = p2a.enter_context(
                        tc.tile_pool(name="psp", bufs=5, space="PSUM"))
                    pop = p2a.enter_context(
                        tc.tile_pool(name="pop", bufs=3, space="PSUM"))
                    exq = p2a.enter_context(tc.tile_pool(name="exq", bufs=6))
                    st2 = p2a.enter_context(tc.tile_pool(name="st2", bufs=8))
                    for b in BS:
                        zero_pads(ofull[b])
                    for b in BS:
                        for h in range(H):
                            tcn, prow = h // 2, (h % 2) * DH
                            exs = []
                            for kc in range(4):
                                sp = psp.tile([128, D], F32, tag="sp",
                                              name=f"sp{b}{h}{kc}")
                                nc.tensor.matmul(
                                    sp[:],
                                    kT[b][prow:prow + DH, tcn,
                                          kc * 128:(kc + 1) * 128],
                                    qT[b][prow:prow + DH, tcn, :],
                                    start=True, stop=True)
                                ex = exq.tile([128, D], BF16, tag="e",
                                              name=f"e{b}{h}{kc}")
                                nc.scalar.activation(ex[:], sp[:], AF.Exp,
                                                     scale=1.0 / np.sqrt(DH))
                                exs.append(ex)
                            opt = pop.tile([128, 4, DH + 2], F32, tag="o",
                                           name=f"op{b}{h}")
                            for qc in range(4):
                                for kc in range(4):
                                    nc.tensor.matmul(
                                        opt[:, qc, :],
                                        exs[kc][:, qc * 128:(qc + 1) * 128],
                                        vaug[b][:, kc, h, :],
                                        start=(kc == 0), stop=(kc == 3))
                            rec = st2.tile([128, 4, 1], F32, tag="r",
                                           name=f"r{b}{h}")
                            nc.vector.reciprocal(rec[:],
                                                 opt[:, :, DH:DH + 1])
                            for qc in range(4):
                                nc.vector.tensor_scalar_mul(
                                    ofull[b][:, qc,
                                             PAD + h * DH:PAD + (h + 1) * DH],
                                    opt[:, qc, 0:DH], rec[:, qc, :])

                # ---- P2b: conv_o, LN1, transpose to hnT ----
                hnT = {b: hnTp.tile([128, 4, D], BF16, tag="h",
                                    name=f"hnT{b}") for b in BS}
                with ExitStack() as p2b:
                    pmm2 = p2b.enter_context(
                        tc.tile_pool(name="pmm2", bufs=6, space="PSUM"))
                    patt = p2b.enter_context(
                        tc.tile_pool(name="patt", bufs=2, space="PSUM"))
                    ypl = p2b.enter_context(tc.tile_pool(name="ypl", bufs=4))
                    lnw2 = p2b.enter_context(tc.tile_pool(name="lnw2", bufs=2))
                    st3 = p2b.enter_context(tc.tile_pool(name="st3", bufs=16))
                    y = {b: ypl.tile([128, 4, D], BF16, tag="y", name=f"y{b}")
                         for b in BS}

                    def wr_y(b, oc, ps):
                        nc.scalar.activation(y[b][:, oc, :], ps[:], AF.Identity,
                                             bias=bpp[:, 8 + oc:9 + oc])
                    conv_std(pmm2, wo_t, ofull, wr_y)

                    # LN1 with eps/2 (absorbed by rsqrt), written in place
                    emit_ln(lnw2, st3, y, y, ln_t["ln1g"], ln_t["ln1b"],
                            padded_src=False)
                    for b in BS:
                        for tcn in range(4):
                            tp = patt.tile([128, 4, 128], BF16, tag="tp",
                                           name=f"tp{b}{tcn}")
                            for cc in range(4):
                                nc.tensor.transpose(
                                    tp[:, cc, :],
                                    y[b][:, cc, tcn * 128:(tcn + 1) * 128],
                                    identb[:])
                            nc.vector.tensor_copy(
                                hnT[b][:, tcn, :],
                                tp[:].rearrange("p c t -> p (c t)"))

            # ---- P3: FFN (all 4 b) ----
            with ExitStack() as fctx:
                w1pool = fctx.enter_context(tc.tile_pool(name="w1pool", bufs=4))
                w2pool = fctx.enter_context(tc.tile_pool(name="w2pool", bufs=16))
                rpool = fctx.enter_context(tc.tile_pool(name="rpool", bufs=4))
                obp = fctx.enter_context(tc.tile_pool(name="obp", bufs=4))
                pff = fctx.enter_context(
                    tc.tile_pool(name="pff", bufs=8, space="PSUM"))

                w1t = []
                for tcn in range(4):
                    t = w1pool.tile([128, DFF], BF16, tag="w1", name=f"w1_{tcn}")
                    nc.sync.dma_start(t[:], wsrc(f"w1_{tcn}"))
                    w1t.append(t)
                w2t = []
                for fc in range(16):
                    t = w2pool.tile([128, D], BF16, tag="w2", name=f"w2_{fc}")
                    nc.sync.dma_start(t[:], wsrc(f"w2_{fc}"))
                    w2t.append(t)

                rl = {b: rpool.tile([128, 16, D], BF16, tag="r", name=f"rl{b}")
                      for b in BS}
                for fc in range(16):
                    ps = {b: pff.tile([128, D], F32, tag="f1",
                                      name=f"f{fc}{b}") for b in BS}
                    for tcn in range(4):
                        lhsT = w1t[tcn][:, fc * 128:(fc + 1) * 128]
                        for b in BS:
                            nc.tensor.matmul(ps[b][:], lhsT, hnT[b][:, tcn, :],
                                             start=(tcn == 0), stop=(tcn == 3))
                    for b in BS:
                        nc.scalar.activation(rl[b][:, fc, :], ps[b][:], AF.Relu,
                                             bias=bpp[:, 12 + fc:13 + fc])
                for cc in range(4):
                    ps2 = {b: pff.tile([128, D], F32, tag="f1",
                                       name=f"g{cc}{b}") for b in BS}
                    for fc in range(16):
                        rhs = w2t[fc][:]
                        for b in BS:
                            nc.tensor.matmul(
                                ps2[b][:],
                                rl[b][:, fc, cc * 128:(cc + 1) * 128], rhs,
                                start=(fc == 0), stop=(fc == 15))
                    for b in BS:
                        ob = obp.tile([128, D], BF16, tag="ob",
                                      name=f"ob{cc}{b}")
                        nc.vector.scalar_tensor_tensor(
                            ob[:], in0=ps2[b][:], scalar=2.0,
                            in1=ln_t["b2r2"][:], op0=ALU.mult, op1=ALU.add)
                        nc.sync.dma_start(
                            outp.ap()[b, cc * 128:(cc + 1) * 128, :], ob[:])

    nc.compile()
    return nc


def prep_in_maps(inputs):
    """Full inputs -> list of 8 per-core input dicts (host-side prep)."""
    f = lambda a: np.ascontiguousarray(np.asarray(a, dtype=np.float32))
    x = f(inputs["x"])
    # per-core x, pre-transposed to [128, 4ci, PAD+D] and causal-padded
    xpad = np.zeros((B, 128, 4, PAD + D), NPBF)
    xpad[:, :, :, PAD:] = x.reshape(B, 4, 128, D).transpose(0, 2, 1, 3) \
        .astype(NPBF)

    wparts = {}
    for n, key in (("win", "w_conv_in"), ("wq", "wq"), ("wk", "wk"),
                   ("wv", "wv"), ("wo", "wo")):
        cw = _conv_w_host(f(inputs[key]))
        for ci in range(4):
            wparts[f"{n}{ci}"] = cw[ci]
    w1 = f(inputs["w1"]).reshape(4, 128, DFF).astype(NPBF)
    for ci in range(4):
        wparts[f"w1_{ci}"] = w1[ci]
    w2 = f(inputs["w2"]).reshape(16, 128, D).astype(NPBF)
    for fc in range(16):
        wparts[f"w2_{fc}"] = w2[fc]

    fb = np.empty(FBLOB, np.float32)
    fparts = {
        "bpp": np.stack(
            [f(inputs["b_conv_in"]).reshape(4, 128)[i] for i in range(4)]
            + [f(inputs["bv"]).reshape(4, 128)[i] for i in range(4)]
            + [f(inputs["bo"]).reshape(4, 128)[i] for i in range(4)]
            + [f(inputs["b1"]).reshape(16, 128)[i] for i in range(16)]
            + [f(inputs["bq"]).reshape(4, 128)[i] for i in range(4)]
            + [f(inputs["bk"]).reshape(4, 128)[i] for i in range(4)],
            axis=1),
        "ln0g": np.tile(f(inputs["ln0_g"]), (128, 1)),
        "ln0b": np.tile(f(inputs["ln0_b"]), (128, 1)),
        "ln1g": np.tile(f(inputs["ln1_g"]), (128, 1)),
        "ln1b": np.tile(f(inputs["ln1_b"]), (128, 1)),
        "bqr": np.tile(f(inputs["bq"]), (128, 1)),
        "bkr": np.tile(f(inputs["bk"]), (128, 1)),
        "b2r2": np.tile(2.0 * f(inputs["b2"]), (128, 1)),
    }
    for n, (off, r, c) in _FREG.items():
        fb[off:off + r * c] = fparts[n].reshape(-1)

    maps = []
    for core in range(NCORES):
        wb = np.zeros(WBLOB, NPBF)
        for n, (off, r, c) in _WREG.items():
            if n.startswith("x") and not n.startswith("xn"):
                b = int(n[1:])
                wb[off:off + r * c] = xpad[core * BL + b].reshape(-1)
            else:
                wb[off:off + r * c] = wparts[n].reshape(-1)
        maps.append({"wblob": wb, "fblob": fb})
    return maps


_NC_CACHE = {}


def get_nc(reps=1):
    if reps not in _NC_CACHE:
        _NC_CACHE[reps] = build_nc(reps)
    return _NC_CACHE[reps]


def kernel(**inputs) -> np.ndarray:
    nc = get_nc()
    in_maps = prep_in_maps(inputs)
    res = run_bass_kernel_spmd(nc, in_maps, list(range(NCORES)))
    return np.concatenate([res.results[c]["outp"] for c in range(NCORES)],
                          axis=0).astype(np.float32)
